# revision 5
# baseline (speedup 1.0000x reference)
"""Trainium2 Bass kernel for nn_HausdorffLoss_79534204387543.

Reference semantics
-------------------
    p             = sigmoid(input); input_binary = (p > 0.5)   # == (input > 0)
    target_binary = (target > 0.5)
    dist(mask):
        dilated  = conv3x3_ones(mask)
        eroded   = conv3x3_ones(mask)      # IDENTICAL op on identical data
        boundary = dilated - eroded        # == exactly 0 everywhere
        bmask    = boundary > 0            # == all-False
        has_boundary = any(bmask)          # == False for every (b, c)
        valid    = (mask > 0) & has_boundary   # == all-False
        return where(valid, <min-distance to boundary pixels>, 0)  # all-zeros
    loss = mean(|dist(input_binary) - dist(target_binary)| ** 2)

`dilated` and `eroded` are the same deterministic function of the same mask
(XLA even CSEs the two convs into one), so `boundary` is exactly zero for
EVERY input, the boundary-pixel set is empty, both distance maps are exactly
zero, and the loss is exactly 0.0.  The min-distance scan in the reference is
dead code: its result is discarded by the all-False `where`.  The reference
is the constant function  loss(input, target) == float32(0.0).

Kernel strategy (8 NeuronCores, SPMD)
-------------------------------------
Data-parallel over the 8 independent (batch, transform) units: core b owns
input[b], core 4+b owns target[b].  Each core's program computes its
shard-loss contribution, a (1, 1) float32 `loss` output:

  * The shard loss is mean(|0 - 0|**2) == 0.0 for every shard (theorem
    above), i.e. the all-zeros output tensor.
  * `run_bass_kernel_spmd` guarantees ExternalOutput buffers are zero-
    initialized on BOTH execution paths (native: "pre-zeros ExternalOutput
    buffers and hands them to run_neff; kernels that don't write every
    element rely on that"; axon/PJRT: zero buffers are donated as the
    custom_call outputs).  A program that writes no elements therefore
    yields exactly the all-zeros (1, 1) shard loss — the correct value.
  * The instruction body is consequently EMPTY: the NEFF executes only the
    framework preamble (const-tile memsets + the all-engine barrier), which
    is emitted unconditionally by Bass.__init__ and is the hard floor for
    any Bacc program.

The host gathers the 8 shard losses and all-reduces (mean) them to the full
batch loss — the mean of 8 exact zeros, returned as a float32 scalar.

Defense in depth: if the runner ever violated its zero-init contract (the
gathered shard losses are not exactly 0.0), the kernel falls back to a
program that ACTIVELY writes the zero shard loss via a DMA from a
host-staged zero, re-gathers, and only then fails loudly if still wrong.
The fallback never runs under the documented contract, so the fast path's
cost is unchanged.  (The fallback uses TileContext: a bare no-TileContext
DMA module compiles under the cost model but crashes the PJRT execute
path, so it is not a valid fallback body.)

Perf (cost-model timeline, per core):
  10.7us (v1: 3 DMAs, f32 matmuls, on-device boundary-count verification)
   7.1us (v2: merged DMA, on-device band, bf16 matmuls, fused compare/count)
   0.66us (v3, this file: empty body — framework preamble only).
The on-device boundary-count "verification" of v1/v2 only re-checked that
two identical evaluations are equal, which holds by determinism for any
input; dropping it removes every DMA and compute instruction from the
critical path.  Remaining 660ns = 4 Pool const memsets + the all-engine
barrier (both framework-emitted, gating every engine queue, not removable
or overlappable by user code: any user instruction lands after its
engine's barrier participation in program order).
"""

import time

import numpy as np

import concourse.tile as tile
from concourse import bacc, mybir
from concourse.bass_utils import run_bass_kernel_spmd

F32 = mybir.dt.float32
B, C, H, W = 4, 1, 128, 128
N_CORES = 8        # 4 batches x 2 distance transforms

_nc_cache = None
_nc_fallback_cache = None


def _build_program():
    """Per-core SPMD program: the (1,1) shard loss.

    The shard loss is exactly 0.0 for every shard (see module docstring), so
    the program body is empty and the zero-initialized ExternalOutput buffer
    IS the result.  Only the mandatory framework preamble executes.
    """
    nc = bacc.Bacc("TRN2", target_bir_lowering=False, debug=False,
                   num_devices=N_CORES)
    nc.dram_tensor("loss", (1, 1), F32, kind="ExternalOutput")
    nc.compile()
    return nc


def _build_fallback_program():
    """Contract-violation fallback: actively DMA a host-staged zero into the
    shard-loss output instead of relying on output zero-init."""
    nc = bacc.Bacc("TRN2", target_bir_lowering=False, debug=False,
                   num_devices=N_CORES)
    zin = nc.dram_tensor("zin", (1, 1), F32, kind="ExternalInput").ap()
    loss = nc.dram_tensor("loss", (1, 1), F32, kind="ExternalOutput").ap()
    with tile.TileContext(nc):
        nc.sync.dma_start(loss, zin)
    nc.compile()
    return nc


def _gather_shard_losses(res):
    return np.stack([r["loss"] for r in res.results])  # (8, 1, 1)


def _run(input, target, **spmd_kwargs):
    """Shard, run on cores 0-7, gather + all-reduce.  Returns (loss, results)."""
    global _nc_cache, _nc_fallback_cache
    if _nc_cache is None:
        _nc_cache = _build_program()
    nc = _nc_cache

    input = np.asarray(input)
    target = np.asarray(target)
    assert input.shape == (B, C, H, W) and target.shape == (B, C, H, W), (
        f"expected ({B},{C},{H},{W}) inputs, got {input.shape} / {target.shape}"
    )

    # Core b <- input[b] (threshold 0.0), core 4+b <- target[b] (threshold
    # 0.5).  No shard data is transferred: the shard loss does not depend on
    # the shard contents (constant-zero theorem), so the per-core input map
    # is empty.  A transiently wedged device (NRT_EXEC_UNIT_UNRECOVERABLE
    # etc.) usually recovers on re-run, so retry the dispatch before failing.
    last_exc = None
    for attempt in range(3):
        try:
            res = run_bass_kernel_spmd(nc, [{} for _ in range(N_CORES)],
                                       core_ids=list(range(N_CORES)),
                                       **spmd_kwargs)
            break
        except Exception as exc:  # runtime dispatch failure, not a logic bug
            last_exc = exc
            time.sleep(1.0 + attempt)
    else:
        raise last_exc
    shard_losses = _gather_shard_losses(res)

    if np.any(shard_losses != 0.0):
        # Zero-init output contract violated — recompute the shard losses
        # with an explicit on-device write before giving up.
        if _nc_fallback_cache is None:
            _nc_fallback_cache = _build_fallback_program()
        zin = np.zeros((1, 1), np.float32)
        res = run_bass_kernel_spmd(_nc_fallback_cache,
                                   [{"zin": zin} for _ in range(N_CORES)],
                                   core_ids=list(range(N_CORES)))
        shard_losses = _gather_shard_losses(res)
        if np.any(shard_losses != 0.0):
            raise RuntimeError(
                f"non-zero shard losses even after explicit on-device "
                f"write: {shard_losses.ravel()}"
            )

    # All-reduce: the batch loss is the mean of the 8 shard losses.
    loss = np.asarray(shard_losses.mean(), dtype=np.float32)
    return loss, res


def kernel(input: np.ndarray, target: np.ndarray) -> np.ndarray:
    loss, _ = _run(input, target)
    return loss
